# revision 7
# baseline (speedup 1.0000x reference)
"""DCN cross-layer stack on 8 Trainium2 NeuronCores (data parallel over batch).

Math: the cross layer x_{l+1} = x_0 * (x_l @ W_i) + b_i + bias_i + x_l keeps
x_l in the form  x_l = x_0 * alpha_l + gamma_l  with alpha_l a per-row scalar
and gamma_l a constant row vector:
    p_i  = x_0 @ W_i                  (per-row, on device)
    q_i  = gamma_i . W_i              (scalar, host — parameter-only)
    alpha_{i+1} = alpha_i*(1+p_i) + q_i
    gamma_{i+1} = gamma_i + (b_i + bias_i)
    out = x_0 * alpha_L + gamma_L

The host passes x twice: natural layout (for the final combine / output) and
transposed (xT, so the PE can contract over d without on-device transposes —
a pure layout change). Device per core (1024 rows): P = x @ W^T via 16 tiny
matmuls with xT chunks stationary, DVE recurrence for alpha, tensor_scalar
combine, store.
"""

import os
from contextlib import ExitStack

import numpy as np

import concourse.bacc as bacc
import concourse.bass as bass
import concourse.tile as tile
from concourse import mybir
from concourse.bass_utils import run_bass_kernel_spmd

FP = mybir.dt.float32

B_FULL = 8192
D = 256
L = 4
N_CORES = 8
B_CORE = B_FULL // N_CORES  # 1024
NT = B_CORE // 128  # 8 row-tiles per core
NG = 2  # recurrence groups
TPG = NT // NG

_cache = {}
last_exec_time_ns = None
last_results = None


def _build_nc(q, zero_gamma):
    """q: tuple of L python floats (q_i). zero_gamma: skip the +gamma add."""
    nc = bacc.Bacc(
        "TRN2", target_bir_lowering=False, debug=False, num_devices=N_CORES
    )
    xT_in = nc.declare_dram_parameter("xT", [D, B_CORE], FP, isOutput=False)
    x_in = nc.declare_dram_parameter("x", [B_CORE, D], FP, isOutput=False)
    wT_in = nc.declare_dram_parameter("wT", [D, L], FP, isOutput=False)
    if not zero_gamma:
        gb_in = nc.declare_dram_parameter("gammab", [128, D], FP, isOutput=False)
    out_ext = nc.declare_dram_parameter("out", [B_CORE, D], FP, isOutput=True)

    with tile.TileContext(nc) as tc, ExitStack() as ctx:
        consts = ctx.enter_context(tc.tile_pool(name="consts", bufs=1))
        xtp = ctx.enter_context(tc.tile_pool(name="xtp", bufs=2))
        xin = ctx.enter_context(tc.tile_pool(name="xin", bufs=2))
        pps = ctx.enter_context(
            tc.tile_pool(name="pps", bufs=1, space=bass.MemorySpace.PSUM)
        )
        apool = ctx.enter_context(tc.tile_pool(name="apool", bufs=NG))
        outp = ctx.enter_context(tc.tile_pool(name="outp", bufs=2))

        # weights first (tiny, needed by the first matmul)
        wT = consts.tile([128, 2, L], FP)
        nc.sync.dma_start(
            out=wT[:], in_=wT_in[:, :].rearrange("(h p) l -> p h l", p=128)
        )
        if not zero_gamma:
            gb = consts.tile([128, D], FP)
            nc.sync.dma_start(out=gb[:], in_=gb_in[:, :])

        # transposed x: four tiles [128, B_CORE//2]: (d-half h, b-chunk c).
        # Issue on two different queue rings so transfers overlap.
        xT_t = {}
        HB = B_CORE // 2
        for h in range(2):
            for c in range(2):
                t_ = xtp.tile([128, HB], FP, tag=f"xT{h}{c}")
                eng = nc.sync if c == 0 else nc.scalar
                eng.dma_start(
                    out=t_[:],
                    in_=xT_in[h * 128 : (h + 1) * 128, c * HB : (c + 1) * HB],
                )
                xT_t[(h, c)] = t_

        # natural x in two batches of 4 row-tiles [128, 4, 256] (gpsimd ring;
        # only needed late, for the final combine)
        x_half = []
        for g in range(NG):
            xh = xin.tile([128, TPG, D], FP, tag=f"x{g}")
            nc.gpsimd.dma_start(
                out=xh[:],
                in_=x_in[g * TPG * 128 : (g + 1) * TPG * 128, :].rearrange(
                    "(t p) d -> p t d", p=128
                ),
            )
            x_half.append(xh)

        # P per group in its own PSUM tensor so the recurrence can start
        # as soon as that group's 8 matmuls are done
        P_g = []
        for g in range(NG):
            P_ps = pps.tile([128, TPG, L], FP, tag=f"P{g}")
            for tt in range(TPG):
                t = g * TPG + tt
                c = t // 4
                sl = slice((t % 4) * 128, (t % 4 + 1) * 128)
                nc.tensor.matmul(
                    P_ps[:, tt, :], xT_t[(0, c)][:, sl], wT[:, 0, :],
                    start=True, stop=False,
                )
                nc.tensor.matmul(
                    P_ps[:, tt, :], xT_t[(1, c)][:, sl], wT[:, 1, :],
                    start=False, stop=True,
                )
            P_g.append(P_ps)

        out_all = []
        for g in range(NG):
            # alpha recurrence on [128, TPG] column groups; reads P from PSUM
            P1 = apool.tile([128, TPG, L], FP, tag="P1")
            nc.vector.tensor_scalar_add(P1[:], P_g[g][:], 1.0)
            a = apool.tile([128, TPG, L - 1], FP, tag="a")
            if q[0] != 0.0:
                nc.vector.tensor_scalar_add(P1[:, :, 0], P1[:, :, 0], q[0])
            src = P1[:, :, 0]
            for i in range(1, L):
                dst = a[:, :, i - 1]
                nc.vector.tensor_mul(dst, src, P1[:, :, i])
                if q[i] != 0.0:
                    nc.vector.tensor_scalar_add(dst, dst, q[i])
                src = dst

            o_g = outp.tile([128, TPG, D], FP, tag=f"o{g}")
            for tt in range(TPG):
                alpha_col = a[:, tt, L - 2 : L - 1]
                x_src = x_half[g][:, tt, :]
                eng = nc.vector if tt % 2 == 0 else nc.scalar
                if zero_gamma:
                    if tt % 2 == 0:
                        nc.vector.tensor_scalar_mul(o_g[:, tt, :], x_src, alpha_col)
                    else:
                        nc.scalar.activation(
                            o_g[:, tt, :],
                            x_src,
                            mybir.ActivationFunctionType.Copy,
                            bias=0.0,
                            scale=alpha_col,
                        )
                else:
                    tmp = outp.tile([128, D], FP, tag="tmp")
                    nc.vector.tensor_scalar_mul(tmp[:], x_src, alpha_col)
                    nc.vector.tensor_add(o_g[:, tt, :], tmp[:], gb[:])
            nc.scalar.dma_start(
                out=out_ext[g * TPG * 128 : (g + 1) * TPG * 128, :].rearrange(
                    "(t p) d -> p t d", p=128
                ),
                in_=o_g[:],
            )
            out_all.append(o_g)
    nc.finalize()
    return nc


def kernel(x, W, b_lin, bias):
    global last_exec_time_ns, last_results
    x = np.ascontiguousarray(x, dtype=np.float32)
    W = np.asarray(W, dtype=np.float32)
    b_lin = np.asarray(b_lin, dtype=np.float32)
    bias = np.asarray(bias, dtype=np.float32)

    # host-side exact collapse of the bias terms (parameter-only precompute)
    c = b_lin[:, None].astype(np.float64) + bias.astype(np.float64)  # [L, D]
    Wd = W.astype(np.float64)
    gamma = np.zeros(D, dtype=np.float64)
    q = np.zeros(L, dtype=np.float64)
    for i in range(L):
        q[i] = float(gamma @ Wd[i])
        gamma = gamma + c[i]
    zero_gamma = not np.any(gamma) and not np.any(q)
    q_f = tuple(float(np.float32(v)) for v in q)

    key = (q_f, zero_gamma)
    if key not in _cache:
        _cache[key] = _build_nc(q_f, zero_gamma)
    nc = _cache[key]

    wT = np.ascontiguousarray(W.T)  # [D, L]
    in_maps = []
    for core in range(N_CORES):
        xs = x[core * B_CORE : (core + 1) * B_CORE]
        m = {
            "x": xs,
            "xT": np.ascontiguousarray(xs.T),
            "wT": wT,
        }
        if not zero_gamma:
            m["gammab"] = np.broadcast_to(
                gamma.astype(np.float32), (128, D)
            ).copy()
        in_maps.append(m)

    trace = bool(os.environ.get("KERNEL_TRACE"))
    res = run_bass_kernel_spmd(nc, in_maps, list(range(N_CORES)), trace=trace)
    last_exec_time_ns = res.exec_time_ns
    last_results = res
    out = np.concatenate([r["out"] for r in res.results], axis=0)
    return out


# revision 9
# speedup vs baseline: 1.0858x; 1.0858x over previous
"""DCN cross-layer stack on 8 Trainium2 NeuronCores (data parallel over batch).

Math: the cross layer x_{l+1} = x_0 * (x_l @ W_i) + b_i + bias_i + x_l keeps
x_l in the form  x_l = x_0 * alpha_l + gamma_l  with alpha_l a per-row scalar
and gamma_l a constant row vector:
    p_i  = x_0 @ W_i                  (per-row, on device)
    q_i  = gamma_i . W_i              (scalar, host — parameter-only)
    alpha_{i+1} = alpha_i*(1+p_i) + q_i
    gamma_{i+1} = gamma_i + (b_i + bias_i)
    out = x_0 * alpha_L + gamma_L

The host passes x twice: natural layout (for the final combine / output) and
transposed (xT, so the PE can contract over d without on-device transposes —
a pure layout change). Device per core (1024 rows): P = x @ W^T via 16 tiny
matmuls with xT chunks stationary, DVE recurrence for alpha, tensor_scalar
combine, store.
"""

import os
from contextlib import ExitStack

import numpy as np

import concourse.bacc as bacc
import concourse.bass as bass
import concourse.tile as tile
from concourse import mybir
from concourse.bass_utils import run_bass_kernel_spmd

FP = mybir.dt.float32

B_FULL = 8192
D = 256
L = 4
N_CORES = 8
B_CORE = B_FULL // N_CORES  # 1024
NT = B_CORE // 128  # 8 row-tiles per core
NG = 2  # recurrence groups
TPG = NT // NG

_cache = {}
last_exec_time_ns = None
last_results = None


def _build_nc(q, zero_gamma):
    """q: tuple of L python floats (q_i). zero_gamma: skip the +gamma add."""
    nc = bacc.Bacc(
        "TRN2", target_bir_lowering=False, debug=False, num_devices=N_CORES
    )
    xT_in = nc.declare_dram_parameter("xT", [D, B_CORE], FP, isOutput=False)
    x_in = nc.declare_dram_parameter("x", [B_CORE, D], FP, isOutput=False)
    wT_in = nc.declare_dram_parameter("wT", [D, L], FP, isOutput=False)
    if not zero_gamma:
        gb_in = nc.declare_dram_parameter("gammab", [128, D], FP, isOutput=False)
    out_ext = nc.declare_dram_parameter("out", [B_CORE, D], FP, isOutput=True)

    with tile.TileContext(nc) as tc, ExitStack() as ctx:
        consts = ctx.enter_context(tc.tile_pool(name="consts", bufs=1))
        xtp = ctx.enter_context(tc.tile_pool(name="xtp", bufs=2))
        xin = ctx.enter_context(tc.tile_pool(name="xin", bufs=2))
        pps = ctx.enter_context(
            tc.tile_pool(name="pps", bufs=1, space=bass.MemorySpace.PSUM)
        )
        apool = ctx.enter_context(tc.tile_pool(name="apool", bufs=NG))
        outp = ctx.enter_context(tc.tile_pool(name="outp", bufs=2))

        # weights first (tiny, needed by the first matmul)
        wT = consts.tile([128, 2, L], FP)
        nc.sync.dma_start(
            out=wT[:], in_=wT_in[:, :].rearrange("(h p) l -> p h l", p=128)
        )
        if not zero_gamma:
            gb = consts.tile([128, D], FP)
            nc.sync.dma_start(out=gb[:], in_=gb_in[:, :])

        # transposed x: 8 chunk tiles [128, 256]: (d-half h, b-chunk c of 2
        # row-tiles). h=0 chunks stream on the SP ring, h=1 on the ACT ring,
        # so matmuls start as soon as the first chunk pair lands and the PE
        # consumption rate tracks the DMA arrival rate.
        NC_CH = NT // 2  # 4 chunks per half
        CW = 256  # chunk width in b columns
        xT_t = {}
        for c in range(NC_CH):
            for h in range(2):
                t_ = xtp.tile([128, CW], FP, tag=f"xT{h}{c}")
                eng = nc.sync if h == 0 else nc.scalar
                eng.dma_start(
                    out=t_[:],
                    in_=xT_in[h * 128 : (h + 1) * 128, c * CW : (c + 1) * CW],
                )
                xT_t[(h, c)] = t_

        # natural x in two batches of 4 row-tiles [128, 4, 256], queued on the
        # same rings BEHIND the xT chunks (only needed late, for the combine)
        x_half = []
        for g in range(NG):
            xh = xin.tile([128, TPG, D], FP, tag=f"x{g}")
            eng = nc.sync if g == 0 else nc.scalar
            eng.dma_start(
                out=xh[:],
                in_=x_in[g * TPG * 128 : (g + 1) * TPG * 128, :].rearrange(
                    "(t p) d -> p t d", p=128
                ),
            )
            x_half.append(xh)

        # P per group in its own PSUM tensor so the recurrence can start
        # as soon as that group's 8 matmuls are done
        P_g = []
        for g in range(NG):
            P_ps = pps.tile([128, TPG, L], FP, tag=f"P{g}")
            for tt in range(TPG):
                t = g * TPG + tt
                c = t // 2
                sl = slice((t % 2) * 128, (t % 2 + 1) * 128)
                nc.tensor.matmul(
                    P_ps[:, tt, :], xT_t[(0, c)][:, sl], wT[:, 0, :],
                    start=True, stop=False,
                )
                nc.tensor.matmul(
                    P_ps[:, tt, :], xT_t[(1, c)][:, sl], wT[:, 1, :],
                    start=False, stop=True,
                )
            P_g.append(P_ps)

        out_all = []
        for g in range(NG):
            # alpha recurrence on [128, TPG] column groups; reads P from PSUM
            P1 = apool.tile([128, TPG, L], FP, tag="P1")
            nc.vector.tensor_scalar_add(P1[:], P_g[g][:], 1.0)
            a = apool.tile([128, TPG, L - 1], FP, tag="a")
            if q[0] != 0.0:
                nc.vector.tensor_scalar_add(P1[:, :, 0], P1[:, :, 0], q[0])
            src = P1[:, :, 0]
            for i in range(1, L):
                dst = a[:, :, i - 1]
                nc.vector.tensor_mul(dst, src, P1[:, :, i])
                if q[i] != 0.0:
                    nc.vector.tensor_scalar_add(dst, dst, q[i])
                src = dst

            o_g = outp.tile([128, TPG, D], FP, tag=f"o{g}")
            for tt in range(TPG):
                alpha_col = a[:, tt, L - 2 : L - 1]
                x_src = x_half[g][:, tt, :]
                eng = nc.vector if tt % 2 == 0 else nc.scalar
                if zero_gamma:
                    if tt % 2 == 0:
                        nc.vector.tensor_scalar_mul(o_g[:, tt, :], x_src, alpha_col)
                    else:
                        nc.scalar.activation(
                            o_g[:, tt, :],
                            x_src,
                            mybir.ActivationFunctionType.Copy,
                            bias=0.0,
                            scale=alpha_col,
                        )
                else:
                    tmp = outp.tile([128, D], FP, tag="tmp")
                    nc.vector.tensor_scalar_mul(tmp[:], x_src, alpha_col)
                    nc.vector.tensor_add(o_g[:, tt, :], tmp[:], gb[:])
            nc.gpsimd.dma_start(
                out=out_ext[g * TPG * 128 : (g + 1) * TPG * 128, :].rearrange(
                    "(t p) d -> p t d", p=128
                ),
                in_=o_g[:],
            )
            out_all.append(o_g)
    nc.finalize()
    return nc


def kernel(x, W, b_lin, bias):
    global last_exec_time_ns, last_results
    x = np.ascontiguousarray(x, dtype=np.float32)
    W = np.asarray(W, dtype=np.float32)
    b_lin = np.asarray(b_lin, dtype=np.float32)
    bias = np.asarray(bias, dtype=np.float32)

    # host-side exact collapse of the bias terms (parameter-only precompute)
    c = b_lin[:, None].astype(np.float64) + bias.astype(np.float64)  # [L, D]
    Wd = W.astype(np.float64)
    gamma = np.zeros(D, dtype=np.float64)
    q = np.zeros(L, dtype=np.float64)
    for i in range(L):
        q[i] = float(gamma @ Wd[i])
        gamma = gamma + c[i]
    zero_gamma = not np.any(gamma) and not np.any(q)
    q_f = tuple(float(np.float32(v)) for v in q)

    key = (q_f, zero_gamma)
    if key not in _cache:
        _cache[key] = _build_nc(q_f, zero_gamma)
    nc = _cache[key]

    wT = np.ascontiguousarray(W.T)  # [D, L]
    in_maps = []
    for core in range(N_CORES):
        xs = x[core * B_CORE : (core + 1) * B_CORE]
        m = {
            "x": xs,
            "xT": np.ascontiguousarray(xs.T),
            "wT": wT,
        }
        if not zero_gamma:
            m["gammab"] = np.broadcast_to(
                gamma.astype(np.float32), (128, D)
            ).copy()
        in_maps.append(m)

    trace = bool(os.environ.get("KERNEL_TRACE"))
    res = run_bass_kernel_spmd(nc, in_maps, list(range(N_CORES)), trace=trace)
    last_exec_time_ns = res.exec_time_ns
    last_results = res
    out = np.concatenate([r["out"] for r in res.results], axis=0)
    return out


# revision 10
# speedup vs baseline: 1.2334x; 1.1359x over previous
"""DCN cross-layer stack on 8 Trainium2 NeuronCores (data parallel over batch).

Math: the cross layer x_{l+1} = x_0 * (x_l @ W_i) + b_i + bias_i + x_l keeps
x_l in the form  x_l = x_0 * alpha_l + gamma_l  with alpha_l a per-row scalar
and gamma_l a constant row vector:
    p_i  = x_0 @ W_i                  (per-row, on device)
    q_i  = gamma_i . W_i              (scalar, host — parameter-only)
    alpha_{i+1} = alpha_i*(1+p_i) + q_i
    gamma_{i+1} = gamma_i + (b_i + bias_i)
    out = x_0 * alpha_L + gamma_L

The host passes x twice: natural layout (for the final combine / output) and
transposed (xT, so the PE can contract over d without on-device transposes —
a pure layout change). Device per core (1024 rows): P = x @ W^T via 16 tiny
matmuls with xT chunks stationary, DVE recurrence for alpha, tensor_scalar
combine, store.
"""

import os
from contextlib import ExitStack

import numpy as np

import concourse.bacc as bacc
import concourse.bass as bass
import concourse.tile as tile
from concourse import mybir
from concourse.bass_utils import run_bass_kernel_spmd

FP = mybir.dt.float32

B_FULL = 8192
D = 256
L = 4
N_CORES = 8
B_CORE = B_FULL // N_CORES  # 1024
NT = B_CORE // 128  # 8 row-tiles per core
NG = 2  # recurrence groups
TPG = NT // NG

_cache = {}
last_exec_time_ns = None
last_results = None


def _build_nc(q, zero_gamma):
    """q: tuple of L python floats (q_i). zero_gamma: skip the +gamma add."""
    nc = bacc.Bacc(
        "TRN2", target_bir_lowering=False, debug=False, num_devices=N_CORES
    )
    xT_in = nc.declare_dram_parameter("xT", [D, B_CORE], FP, isOutput=False)
    x_in = nc.declare_dram_parameter("x", [B_CORE, D], FP, isOutput=False)
    wT_in = nc.declare_dram_parameter("wT", [D, L], FP, isOutput=False)
    if not zero_gamma:
        gb_in = nc.declare_dram_parameter("gammab", [128, D], FP, isOutput=False)
    out_ext = nc.declare_dram_parameter("out", [B_CORE, D], FP, isOutput=True)

    with tile.TileContext(nc) as tc, ExitStack() as ctx:
        consts = ctx.enter_context(tc.tile_pool(name="consts", bufs=1))
        xtp = ctx.enter_context(tc.tile_pool(name="xtp", bufs=2))
        xin = ctx.enter_context(tc.tile_pool(name="xin", bufs=2))
        pps = ctx.enter_context(
            tc.tile_pool(name="pps", bufs=1, space=bass.MemorySpace.PSUM)
        )
        apool = ctx.enter_context(tc.tile_pool(name="apool", bufs=NG))
        outp = ctx.enter_context(tc.tile_pool(name="outp", bufs=2))

        # weights first (tiny, needed by the first matmul)
        wT = consts.tile([128, 2, L], FP)
        nc.sync.dma_start(
            out=wT[:], in_=wT_in[:, :].rearrange("(h p) l -> p h l", p=128)
        )
        if not zero_gamma:
            gb = consts.tile([128, D], FP)
            nc.sync.dma_start(out=gb[:], in_=gb_in[:, :])

        # transposed x: 8 chunk tiles [128, 256]: (d-half h, b-chunk c of 2
        # row-tiles). h=0 chunks stream on the SP ring, h=1 on the ACT ring,
        # so matmuls start as soon as the first chunk pair lands and the PE
        # consumption rate tracks the DMA arrival rate.
        NC_CH = NT // 2  # 4 chunks per half
        CW = 256  # chunk width in b columns
        xT_t = {}
        for c in range(NC_CH):
            for h in range(2):
                t_ = xtp.tile([128, CW], FP, tag=f"xT{h}{c}")
                eng = nc.sync if c < 2 else nc.scalar
                eng.dma_start(
                    out=t_[:],
                    in_=xT_in[h * 128 : (h + 1) * 128, c * CW : (c + 1) * CW],
                )
                xT_t[(h, c)] = t_

        # natural x in two batches of 4 row-tiles [128, 4, 256], queued on the
        # same rings BEHIND the xT chunks (only needed late, for the combine)
        x_half = []
        for g in range(NG):
            xh = xin.tile([128, TPG, D], FP, tag=f"x{g}")
            eng = nc.sync if g == 0 else nc.scalar
            eng.dma_start(
                out=xh[:],
                in_=x_in[g * TPG * 128 : (g + 1) * TPG * 128, :].rearrange(
                    "(t p) d -> p t d", p=128
                ),
            )
            x_half.append(xh)

        # P per group in its own PSUM tensor so the recurrence can start
        # as soon as that group's 8 matmuls are done
        P_g = []
        for g in range(NG):
            P_ps = pps.tile([128, TPG, L], FP, tag=f"P{g}")
            for tt in range(TPG):
                t = g * TPG + tt
                c = t // 2
                sl = slice((t % 2) * 128, (t % 2 + 1) * 128)
                nc.tensor.matmul(
                    P_ps[:, tt, :], xT_t[(0, c)][:, sl], wT[:, 0, :],
                    start=True, stop=False,
                )
                nc.tensor.matmul(
                    P_ps[:, tt, :], xT_t[(1, c)][:, sl], wT[:, 1, :],
                    start=False, stop=True,
                )
            P_g.append(P_ps)

        out_all = []
        for g in range(NG):
            # alpha recurrence: a_i = (P_i + 1) * a_{i-1} (+ q_i), with
            # P read straight from PSUM and the +1 fused into each op
            a = apool.tile([128, TPG, L], FP, tag="a")
            nc.vector.tensor_scalar_add(a[:, :, 0], P_g[g][:, :, 0], 1.0 + q[0])
            src = a[:, :, 0]
            for i in range(1, L):
                dst = a[:, :, i]
                nc.vector.scalar_tensor_tensor(
                    dst,
                    P_g[g][:, :, i],
                    1.0,
                    src,
                    op0=mybir.AluOpType.add,
                    op1=mybir.AluOpType.mult,
                )
                if q[i] != 0.0:
                    nc.vector.tensor_scalar_add(dst, dst, q[i])
                src = dst

            o_g = outp.tile([128, TPG, D], FP, tag=f"o{g}")
            for tt in range(TPG):
                alpha_col = a[:, tt, L - 1 : L]
                x_src = x_half[g][:, tt, :]
                eng = nc.vector if tt % 2 == 0 else nc.scalar
                if zero_gamma:
                    if tt % 2 == 0:
                        nc.vector.tensor_scalar_mul(o_g[:, tt, :], x_src, alpha_col)
                    else:
                        nc.scalar.activation(
                            o_g[:, tt, :],
                            x_src,
                            mybir.ActivationFunctionType.Copy,
                            bias=0.0,
                            scale=alpha_col,
                        )
                else:
                    tmp = outp.tile([128, D], FP, tag="tmp")
                    nc.vector.tensor_scalar_mul(tmp[:], x_src, alpha_col)
                    nc.vector.tensor_add(o_g[:, tt, :], tmp[:], gb[:])
            nc.gpsimd.dma_start(
                out=out_ext[g * TPG * 128 : (g + 1) * TPG * 128, :].rearrange(
                    "(t p) d -> p t d", p=128
                ),
                in_=o_g[:],
            )
            out_all.append(o_g)
    nc.finalize()
    return nc


def kernel(x, W, b_lin, bias):
    global last_exec_time_ns, last_results
    x = np.ascontiguousarray(x, dtype=np.float32)
    W = np.asarray(W, dtype=np.float32)
    b_lin = np.asarray(b_lin, dtype=np.float32)
    bias = np.asarray(bias, dtype=np.float32)

    # host-side exact collapse of the bias terms (parameter-only precompute)
    c = b_lin[:, None].astype(np.float64) + bias.astype(np.float64)  # [L, D]
    Wd = W.astype(np.float64)
    gamma = np.zeros(D, dtype=np.float64)
    q = np.zeros(L, dtype=np.float64)
    for i in range(L):
        q[i] = float(gamma @ Wd[i])
        gamma = gamma + c[i]
    zero_gamma = not np.any(gamma) and not np.any(q)
    q_f = tuple(float(np.float32(v)) for v in q)

    key = (q_f, zero_gamma)
    if key not in _cache:
        _cache[key] = _build_nc(q_f, zero_gamma)
    nc = _cache[key]

    wT = np.ascontiguousarray(W.T)  # [D, L]
    in_maps = []
    for core in range(N_CORES):
        xs = x[core * B_CORE : (core + 1) * B_CORE]
        m = {
            "x": xs,
            "xT": np.ascontiguousarray(xs.T),
            "wT": wT,
        }
        if not zero_gamma:
            m["gammab"] = np.broadcast_to(
                gamma.astype(np.float32), (128, D)
            ).copy()
        in_maps.append(m)

    trace = bool(os.environ.get("KERNEL_TRACE"))
    res = run_bass_kernel_spmd(nc, in_maps, list(range(N_CORES)), trace=trace)
    last_exec_time_ns = res.exec_time_ns
    last_results = res
    out = np.concatenate([r["out"] for r in res.results], axis=0)
    return out
